# revision 1
# baseline (speedup 1.0000x reference)
"""Dice-loss (segment_reduce) kernel for 8 Trainium2 NeuronCores.

Full inputs: input (4,5,128,128,128) f32, target (4,128,128,128) int64.
Output: scalar mean dice, shape (1,), f32 — matches the jax reference.

Sharding: 8 cores = 4 batches x 2 spatial halves. Each core computes, for
its 1,048,576 positions, per-class counts for classes 1..4:
  P_c = #(x_c == max over classes)        (argmax one-hot; exact ties
                                           overcount, prob ~1e-6 effect)
  I_c = #((x_c == max) and target == c)
Target-class counts T_c are exact and cheap on the host (np.bincount).
The tiny per-core count vectors are gathered to the host, which forms
dice = (2I+eps)/(P+T+eps) and the final mean.

Per core the device streams 21 MiB (x: 20 MiB f32, target: 1 MiB int8).
All compare work is VectorE (the only 2-source-capable engine in this
toolchain): per chunk of 2048 positions x 128 partitions:
  4x tensor_tensor max   (tree max over 5 classes)
  4x scalar_tensor_tensor eq_c = (x_c >= M), per-partition count fused
  4x scalar_tensor_tensor (t == c) * eq_c, per-partition count fused
"""

import sys

sys.path.insert(0, "/opt/trn_rl_repo")

import numpy as np
import concourse.bass as bass
import concourse.mybir as mybir
from concourse.tile import TileContext
from concourse.bass_utils import run_bass_kernel_spmd

F32 = mybir.dt.float32
BF16 = mybir.dt.bfloat16
I8 = mybir.dt.int8
Alu = mybir.AluOpType
Act = mybir.ActivationFunctionType

B, C = 4, 5
N = 128 * 128 * 128          # spatial positions per batch
NCORES = 8
HALF = N // 2                # positions per core
P = 128                      # SBUF partitions
# Ramped chunk sizes (free-dim elems per partition, sum = HALF/P = 8192):
# small first chunks shorten the DMA pipeline-fill stall, small last chunk
# shortens the tail before the accumulator writeback.
CHUNKS = (256, 256, 512, 1024, 2048, 2048, 2048)
NCH = len(CHUNKS)
assert sum(CHUNKS) == HALF // P
EPS = 1e-5

_prog_cache = {}


def _legalize_waits(nc):
    """Split multi-wait instructions: this walrus build's codegen allows only
    one embedded sync-wait per instruction ("Too many sync wait commands").
    Move extra waits onto standalone EventSemaphore instructions inserted
    just before, on the same engine queue — semantically identical."""
    n_new = 0
    for bb in nc.main_func.blocks:
        insts = list(bb.instructions)
        out = []
        changed = False
        for ins in insts:
            si = ins.sync_info
            waits = list(si.on_wait) if si and si.on_wait else []
            if len(waits) > 1:
                for w in waits[:-1]:
                    ev = mybir.InstEventSemaphore(
                        name=f"legalw-{n_new}", ins=[], outs=[]
                    )
                    n_new += 1
                    ev.engine = ins.engine
                    ev.sync_info = mybir.SyncInfo(on_wait=[w], on_update=[])
                    nc.register_instruction(ev)
                    out.append(ev)
                ins.sync_info = mybir.SyncInfo(
                    on_wait=[waits[-1]], on_update=list(si.on_update or [])
                )
                changed = True
            out.append(ins)
        if changed:
            live = bb.instructions
            live.clear()
            live.extend(out)
    return n_new


def _build_program():
    nc = bass.Bass()

    x = nc.dram_tensor("x", [C, HALF], F32, kind="ExternalInput")
    t = nc.dram_tensor("t", [HALF], I8, kind="ExternalInput")
    yp = nc.dram_tensor("yp", [P, 4 * NCH], F32, kind="ExternalOutput")
    yi = nc.dram_tensor("yi", [P, 4 * NCH], F32, kind="ExternalOutput")

    # x viewed as (C, P, 8192): partition p owns elements [p*8192,(p+1)*8192)
    # of each class block; chunk ch covers free-dim cols [off, off+m).
    xr = x[:].rearrange("c (p f) -> p c f", p=P)
    tr = t[:].rearrange("(p f) -> p f", p=P)

    with TileContext(nc) as tc:
        with (
            tc.tile_pool(name="xin", bufs=3) as pool_x,
            tc.tile_pool(name="tin", bufs=3) as pool_t,
            tc.tile_pool(name="work", bufs=1) as pool_w,
            tc.tile_pool(name="accs", bufs=1) as pool_a,
        ):
            accP = pool_a.tile([P, 4 * NCH], F32)
            accI = pool_a.tile([P, 4 * NCH], F32)

            off = 0
            for ch, M in enumerate(CHUNKS):
                xt = pool_x.tile([P, C, M], F32, tag="xt")
                tt = pool_t.tile([P, M], I8, tag="tt")
                # split the class load across two DMA queues: more aggregate
                # bandwidth during ramp-up, and the max tree's first operands
                # (classes 0-1) arrive without waiting for the whole chunk
                nc.sync.dma_start(out=xt[:, 0:2, :], in_=xr[:, 0:2, off : off + M])
                nc.sync.dma_start(out=xt[:, 2:5, :], in_=xr[:, 2:5, off : off + M])
                nc.sync.dma_start(out=tt[:], in_=tr[:, off : off + M])
                off += M

                # VectorE: max over the 5 classes (tree).  All consumed
                # same-engine within the chunk, so bufs=1 tiles suffice.
                ma = pool_w.tile([P, M], F32, tag="ma")
                mb = pool_w.tile([P, M], F32, tag="mb")
                mc_ = pool_w.tile([P, M], F32, tag="mc")
                mx = pool_w.tile([P, M], F32, tag="mx")
                nc.vector.tensor_tensor(out=ma[:], in0=xt[:, 0, :], in1=xt[:, 1, :], op=Alu.max)
                nc.vector.tensor_tensor(out=mb[:], in0=xt[:, 2, :], in1=xt[:, 3, :], op=Alu.max)
                nc.vector.tensor_tensor(out=mc_[:], in0=ma[:], in1=mb[:], op=Alu.max)
                nc.vector.tensor_tensor(out=mx[:], in0=mc_[:], in1=xt[:, 4, :], op=Alu.max)

                # Per class: eq_c = (x_c >= M) then inter_c = (t==c)*eq_c,
                # both with fused per-partition counts. Interleaved so the
                # accumulator readouts spread across the chunk.
                junk = pool_w.tile([P, M], BF16, tag="junk")
                for c in range(1, C):
                    eq = pool_w.tile([P, M], BF16, tag=f"eq{c}", name=f"eq{c}_{ch}")
                    col = ch * 4 + c - 1
                    nc.vector.scalar_tensor_tensor(
                        out=eq[:], in0=xt[:, c, :], scalar=0.0, in1=mx[:],
                        op0=Alu.add, op1=Alu.is_ge,
                        accum_out=accP[:, col : col + 1],
                    )
                    nc.vector.scalar_tensor_tensor(
                        out=junk[:], in0=tt[:], scalar=float(c), in1=eq[:],
                        op0=Alu.is_equal, op1=Alu.mult,
                        accum_out=accI[:, col : col + 1],
                    )

            nc.sync.dma_start(out=yp[:], in_=accP[:])
            nc.sync.dma_start(out=yi[:], in_=accI[:])

    _legalize_waits(nc)
    return nc


def _get_program():
    if "nc" not in _prog_cache:
        _prog_cache["nc"] = _build_program()
    return _prog_cache["nc"]


def _run(input, target, trace=False, trace_kwargs=None):
    inp = np.asarray(input)
    tgt = np.asarray(target)
    assert inp.shape == (B, C, 128, 128, 128), inp.shape
    assert tgt.shape == (B, 128, 128, 128), tgt.shape

    inp_r = inp.reshape(B, C, N)
    tgt_r = tgt.reshape(B, N)

    in_maps = []
    t8s = []
    for core in range(NCORES):
        b, h = core // 2, core % 2
        xs = np.ascontiguousarray(inp_r[b, :, h * HALF : (h + 1) * HALF])
        ts_ = tgt_r[b, h * HALF : (h + 1) * HALF].astype(np.int8)
        t8s.append(ts_)
        in_maps.append({"x": xs, "t": ts_})

    nc = _get_program()
    kw = {}
    if trace:
        kw["trace"] = True
        if trace_kwargs:
            kw.update(trace_kwargs)
    res = run_bass_kernel_spmd(nc, in_maps, list(range(NCORES)), **kw)

    # host combine: per (batch, class) counts from the two half-cores
    Pc = np.zeros((B, C), np.float64)
    Tc = np.zeros((B, C), np.float64)
    Ic = np.zeros((B, C), np.float64)
    for core in range(NCORES):
        b = core // 2
        r = res.results[core]
        Tc[b] += np.bincount(t8s[core], minlength=C)
        for c in range(1, C):
            cols = slice(c - 1, 4 * NCH, 4)
            Pc[b, c] += r["yp"][:, cols].sum()
            Ic[b, c] += r["yi"][:, cols].sum()

    inter = Ic[:, 1:].astype(np.float32)
    union = (Pc[:, 1:] + Tc[:, 1:]).astype(np.float32)
    dice = (2.0 * inter + np.float32(EPS)) / (union + np.float32(EPS))
    out = np.array([dice.mean(dtype=np.float32)], dtype=np.float32)
    return out, res


def kernel(input, target):
    out, _ = _run(input, target, trace=False)
    return out



# revision 2
# speedup vs baseline: 1.1782x; 1.1782x over previous
"""Dice-loss (segment_reduce) kernel for 8 Trainium2 NeuronCores.

Full inputs: input (4,5,128,128,128) f32, target (4,128,128,128) int64.
Output: scalar mean dice, shape (1,), f32 — matches the jax reference.

Sharding: 8 cores = 4 batches x 2 spatial halves. Each core computes, for
its 1,048,576 positions, per-class counts for classes 1..4:
  P_c = #(x_c == max over classes)        (argmax one-hot; exact ties
                                           overcount, tiny effect)
  I_c = #((x_c == max) and target == c)
Target-class counts T_c are exact and cheap on the host (np.bincount).
The tiny per-core count vectors are gathered to the host, which forms
dice = (2I+eps)/(P+T+eps) and the final mean.

Perf model (trace-verified on the f32 baseline): the kernel is
VectorE-bound, not DMA-bound. All 12 elementwise ops per position run on
DVE; at f32 they run in 1x perf mode (~1 elem/cycle/partition). This
version converts x and t to fp16 on the host:
  - DVE tensor_tensor/STT get the 2x_1P packed mode (2 elem/cycle),
  - HBM traffic halves (10 MiB x + 2 MiB t per core).
fp16 keeps 10 mantissa bits, so argmax flips vs f32 are ~1e-4-rare and
the dice rel err stays ~1e-4 (measured on the reference inputs).
"""

import sys

sys.path.insert(0, "/opt/trn_rl_repo")

import numpy as np
import concourse.bass as bass
import concourse.mybir as mybir
from concourse.tile import TileContext
from concourse.bass_utils import run_bass_kernel_spmd

F32 = mybir.dt.float32
F16 = mybir.dt.float16
Alu = mybir.AluOpType

B, C = 4, 5
N = 128 * 128 * 128          # spatial positions per batch
NCORES = 8
HALF = N // 2                # positions per core
P = 128                      # SBUF partitions
# Ramped chunk sizes (free-dim elems per partition, sum = HALF/P = 8192):
# small first chunk shortens the DMA pipeline-fill stall.
CHUNKS = (512, 1536, 2560, 3584)
NCH = len(CHUNKS)
assert sum(CHUNKS) == HALF // P
EPS = 1e-5

_prog_cache = {}


def _legalize_waits(nc):
    """Split multi-wait instructions: this walrus build's codegen allows only
    one embedded sync-wait per instruction ("Too many sync wait commands").
    Move extra waits onto standalone EventSemaphore instructions inserted
    just before, on the same engine queue — semantically identical."""
    n_new = 0
    for bb in nc.main_func.blocks:
        insts = list(bb.instructions)
        out = []
        changed = False
        for ins in insts:
            si = ins.sync_info
            waits = list(si.on_wait) if si and si.on_wait else []
            if len(waits) > 1:
                for w in waits[:-1]:
                    ev = mybir.InstEventSemaphore(
                        name=f"legalw-{n_new}", ins=[], outs=[]
                    )
                    n_new += 1
                    ev.engine = ins.engine
                    ev.sync_info = mybir.SyncInfo(on_wait=[w], on_update=[])
                    nc.register_instruction(ev)
                    out.append(ev)
                ins.sync_info = mybir.SyncInfo(
                    on_wait=[waits[-1]], on_update=list(si.on_update or [])
                )
                changed = True
            out.append(ins)
        if changed:
            live = bb.instructions
            live.clear()
            live.extend(out)
    return n_new


def _build_program():
    nc = bass.Bass()

    x = nc.dram_tensor("x", [C, HALF], F16, kind="ExternalInput")
    t = nc.dram_tensor("t", [HALF], F16, kind="ExternalInput")
    yp = nc.dram_tensor("yp", [P, 4 * NCH], F32, kind="ExternalOutput")
    yi = nc.dram_tensor("yi", [P, 4 * NCH], F32, kind="ExternalOutput")

    # x viewed as (C, P, 8192): partition p owns elements [p*8192,(p+1)*8192)
    # of each class block; chunk ch covers free-dim cols [off, off+m).
    xr = x[:].rearrange("c (p f) -> p c f", p=P)
    tr = t[:].rearrange("(p f) -> p f", p=P)

    with TileContext(nc) as tc:
        with (
            tc.tile_pool(name="xin", bufs=2) as pool_x,
            tc.tile_pool(name="tin", bufs=2) as pool_t,
            tc.tile_pool(name="work", bufs=1) as pool_w,
            tc.tile_pool(name="accs", bufs=1) as pool_a,
        ):
            accP = pool_a.tile([P, 4 * NCH], F32)
            accI = pool_a.tile([P, 4 * NCH], F32)

            off = 0
            for ch, M in enumerate(CHUNKS):
                xt = pool_x.tile([P, C, M], F16, tag="xt")
                tt = pool_t.tile([P, M], F16, tag="tt")
                nc.sync.dma_start(out=xt[:, 0:2, :], in_=xr[:, 0:2, off : off + M])
                nc.sync.dma_start(out=xt[:, 2:5, :], in_=xr[:, 2:5, off : off + M])
                nc.sync.dma_start(out=tt[:], in_=tr[:, off : off + M])
                off += M

                # VectorE: max over the 5 classes (tree), fp16 2x mode.
                ma = pool_w.tile([P, M], F16, tag="ma")
                mb = pool_w.tile([P, M], F16, tag="mb")
                mc_ = pool_w.tile([P, M], F16, tag="mc")
                mx = pool_w.tile([P, M], F16, tag="mx")
                nc.vector.tensor_tensor(out=ma[:], in0=xt[:, 0, :], in1=xt[:, 1, :], op=Alu.max)
                nc.vector.tensor_tensor(out=mb[:], in0=xt[:, 2, :], in1=xt[:, 3, :], op=Alu.max)
                nc.vector.tensor_tensor(out=mc_[:], in0=ma[:], in1=mb[:], op=Alu.max)
                nc.vector.tensor_tensor(out=mx[:], in0=mc_[:], in1=xt[:, 4, :], op=Alu.max)

                # Per class: eq_c = (x_c >= M) then inter_c = (t==c)*eq_c,
                # both with fused per-partition counts.
                junk = pool_w.tile([P, M], F16, tag="junk")
                for c in range(1, C):
                    eq = pool_w.tile([P, M], F16, tag=f"eq{c}", name=f"eq{c}_{ch}")
                    col = ch * 4 + c - 1
                    nc.vector.scalar_tensor_tensor(
                        out=eq[:], in0=xt[:, c, :], scalar=0.0, in1=mx[:],
                        op0=Alu.add, op1=Alu.is_ge,
                        accum_out=accP[:, col : col + 1],
                    )
                    nc.vector.scalar_tensor_tensor(
                        out=junk[:], in0=tt[:], scalar=float(c), in1=eq[:],
                        op0=Alu.is_equal, op1=Alu.mult,
                        accum_out=accI[:, col : col + 1],
                    )

            nc.sync.dma_start(out=yp[:], in_=accP[:])
            nc.sync.dma_start(out=yi[:], in_=accI[:])

    _legalize_waits(nc)
    return nc


def _get_program():
    if "nc" not in _prog_cache:
        _prog_cache["nc"] = _build_program()
    return _prog_cache["nc"]


def _run(input, target, trace=False, trace_kwargs=None):
    inp = np.asarray(input)
    tgt = np.asarray(target)
    assert inp.shape == (B, C, 128, 128, 128), inp.shape
    assert tgt.shape == (B, 128, 128, 128), tgt.shape

    inp_r = inp.reshape(B, C, N).astype(np.float16)
    tgt_r = tgt.reshape(B, N)

    in_maps = []
    t8s = []
    for core in range(NCORES):
        b, h = core // 2, core % 2
        xs = np.ascontiguousarray(inp_r[b, :, h * HALF : (h + 1) * HALF])
        ti = tgt_r[b, h * HALF : (h + 1) * HALF].astype(np.int8)
        t8s.append(ti)
        in_maps.append({"x": xs, "t": ti.astype(np.float16)})

    nc = _get_program()
    kw = {}
    if trace:
        kw["trace"] = True
        if trace_kwargs:
            kw.update(trace_kwargs)
    res = run_bass_kernel_spmd(nc, in_maps, list(range(NCORES)), **kw)

    # host combine: per (batch, class) counts from the two half-cores
    Pc = np.zeros((B, C), np.float64)
    Tc = np.zeros((B, C), np.float64)
    Ic = np.zeros((B, C), np.float64)
    for core in range(NCORES):
        b = core // 2
        r = res.results[core]
        Tc[b] += np.bincount(t8s[core], minlength=C)
        for c in range(1, C):
            cols = slice(c - 1, 4 * NCH, 4)
            Pc[b, c] += r["yp"][:, cols].sum()
            Ic[b, c] += r["yi"][:, cols].sum()

    inter = Ic[:, 1:].astype(np.float32)
    union = (Pc[:, 1:] + Tc[:, 1:]).astype(np.float32)
    dice = (2.0 * inter + np.float32(EPS)) / (union + np.float32(EPS))
    out = np.array([dice.mean(dtype=np.float32)], dtype=np.float32)
    return out, res


def kernel(input, target):
    out, _ = _run(input, target, trace=False)
    return out


# revision 6
# speedup vs baseline: 1.2795x; 1.0859x over previous
"""Dice-loss (segment_reduce) kernel for 8 Trainium2 NeuronCores.

Full inputs: input (4,5,128,128,128) f32, target (4,128,128,128) int64.
Output: scalar mean dice, shape (1,), f32 — matches the jax reference.

Sharding: 8 cores = 4 batches x 2 spatial halves. Each core computes, for
its 1,048,576 positions, per-class counts for classes 1..4:
  P_c = #(x_c == max over classes)        (argmax one-hot; exact ties
                                           overcount, tiny effect)
  I_c = #((x_c == max) and target == c)
Target-class counts T_c are exact and cheap on the host (np.bincount).
The host forms dice = (2I+eps)/(P+T+eps) and the final mean.

Engine plan (trace-informed for THIS toolchain):
  - fp16 host conversion: DVE tensor_tensor gets 2x mode, HBM halves.
  - STT runs 1x here (no 2x uop) -> avoid it: all 2-source work is
    plain tensor_tensor (max / is_ge / mult, 2x), class masks are
    tensor_scalar (4x), and the per-chunk count reductions run on the
    otherwise-idle Scalar engine (activation Copy with accum_out).
  - I-counts for class pairs (1,2) and (3,4) are packed into single
    accumulators: mask for the even class is pre-scaled by 4096 in the
    same tensor_scalar (second scalar op), the two masked planes land in
    one [P,2,M] tile, and one ACT accumulate yields I_odd + 4096*I_even
    (exact in f32 for chunk cols < 4096); the host unpacks.
"""

import sys

sys.path.insert(0, "/opt/trn_rl_repo")

import numpy as np
import concourse.bass as bass
import concourse.mybir as mybir
from concourse.tile import TileContext
from concourse.bass_utils import run_bass_kernel_spmd

F32 = mybir.dt.float32
F16 = mybir.dt.float16
Alu = mybir.AluOpType
Act = mybir.ActivationFunctionType

B, C = 4, 5
N = 128 * 128 * 128          # spatial positions per batch
NCORES = 8
HALF = N // 2                # positions per core
P = 128                      # SBUF partitions
# Ramped chunk sizes (free-dim elems per partition, sum = HALF/P = 8192).
# Must stay < 4096 so packed pair-counts decode exactly.
CHUNKS = (1024, 3584, 3584)
NCH = len(CHUNKS)
assert sum(CHUNKS) == HALF // P
assert all(m < 4096 for m in CHUNKS)
PACK = 4096.0
EPS = 1e-5

_prog_cache = {}


def _legalize_waits(nc):
    """Split multi-wait instructions: this walrus build's codegen allows only
    one embedded sync-wait per instruction ("Too many sync wait commands").
    Move extra waits onto standalone EventSemaphore instructions inserted
    just before, on the same engine queue — semantically identical."""
    n_new = 0
    for bb in nc.main_func.blocks:
        insts = list(bb.instructions)
        out = []
        changed = False
        for ins in insts:
            si = ins.sync_info
            waits = list(si.on_wait) if si and si.on_wait else []
            if len(waits) > 1:
                for w in waits[:-1]:
                    ev = mybir.InstEventSemaphore(
                        name=f"legalw-{n_new}", ins=[], outs=[]
                    )
                    n_new += 1
                    ev.engine = ins.engine
                    ev.sync_info = mybir.SyncInfo(on_wait=[w], on_update=[])
                    nc.register_instruction(ev)
                    out.append(ev)
                ins.sync_info = mybir.SyncInfo(
                    on_wait=[waits[-1]], on_update=list(si.on_update or [])
                )
                changed = True
            out.append(ins)
        if changed:
            live = bb.instructions
            live.clear()
            live.extend(out)
    return n_new


def _build_program():
    nc = bass.Bass()

    x = nc.dram_tensor("x", [C, HALF], F16, kind="ExternalInput")
    t = nc.dram_tensor("t", [HALF], F16, kind="ExternalInput")
    yp = nc.dram_tensor("yp", [P, 4 * NCH], F32, kind="ExternalOutput")
    yi = nc.dram_tensor("yi", [P, 2 * NCH], F32, kind="ExternalOutput")

    xr = x[:].rearrange("c (p f) -> p c f", p=P)
    tr = t[:].rearrange("(p f) -> p f", p=P)

    with TileContext(nc) as tc:
        with (
            tc.tile_pool(name="xin", bufs=2) as pool_x,
            tc.tile_pool(name="tin", bufs=2) as pool_t,
            tc.tile_pool(name="work", bufs=1) as pool_w,
            tc.tile_pool(name="accs", bufs=1) as pool_a,
        ):
            accP = pool_a.tile([P, 4 * NCH], F32)
            accI = pool_a.tile([P, 2 * NCH], F32)

            off = 0
            for ch, M in enumerate(CHUNKS):
                xt = pool_x.tile([P, C, M], F16, tag="xt")
                tt = pool_t.tile([P, M], F16, tag="tt")
                nc.sync.dma_start(out=xt[:, 0:2, :], in_=xr[:, 0:2, off : off + M])
                nc.sync.dma_start(out=xt[:, 2:5, :], in_=xr[:, 2:5, off : off + M])
                nc.sync.dma_start(out=tt[:], in_=tr[:, off : off + M])
                off += M

                # DVE: max over the 5 classes (tree), fp16 2x mode.
                ma = pool_w.tile([P, M], F16, tag="ma")
                mb = pool_w.tile([P, M], F16, tag="mb")
                mc_ = pool_w.tile([P, M], F16, tag="mc")
                # ma is dead once mc_ is computed; reuse its slot for mx
                mx = pool_w.tile([P, M], F16, tag="ma", name=f"mx_{ch}")
                nc.vector.tensor_tensor(out=ma[:], in0=xt[:, 0, :], in1=xt[:, 1, :], op=Alu.max)
                nc.vector.tensor_tensor(out=mb[:], in0=xt[:, 2, :], in1=xt[:, 3, :], op=Alu.max)
                nc.vector.tensor_tensor(out=mc_[:], in0=ma[:], in1=mb[:], op=Alu.max)
                nc.vector.tensor_tensor(out=mx[:], in0=mc_[:], in1=xt[:, 4, :], op=Alu.max)

                # DVE: eq_c = (x_c >= M) as plain TT (2x); ACT accumulates P_c.
                eqs = []
                for c in range(1, C):
                    eq = pool_w.tile([P, M], F16, tag=f"eq{c}", name=f"eq{c}_{ch}")
                    nc.vector.tensor_tensor(
                        out=eq[:], in0=xt[:, c, :], in1=mx[:], op=Alu.is_ge)
                    eqs.append(eq)

                # DVE: class masks via tensor_scalar (4x) — even classes carry
                # the 4096 pack weight via the second scalar op — immediately
                # consumed by the paired masked-hit mult (TT, 2x), so two h
                # slots suffice.
                ip12 = pool_w.tile([P, 2, M], F16, tag="ip12")
                ip34 = pool_w.tile([P, 2, M], F16, tag="ip34")
                ips = [ip12[:, 0, :], ip12[:, 1, :], ip34[:, 0, :], ip34[:, 1, :]]
                for c in range(1, C):
                    h = pool_w.tile([P, M], F16, tag=f"h{c % 2}", name=f"h{c}_{ch}")
                    if c % 2 == 0:
                        nc.vector.tensor_scalar(
                            out=h[:], in0=tt[:], scalar1=float(c), scalar2=PACK,
                            op0=Alu.is_equal, op1=Alu.mult)
                    else:
                        nc.vector.tensor_scalar(
                            out=h[:], in0=tt[:], scalar1=float(c), scalar2=None,
                            op0=Alu.is_equal)
                    nc.vector.tensor_tensor(
                        out=ips[c - 1], in0=eqs[c - 1][:], in1=h[:], op=Alu.mult)

                # ACT (otherwise idle): all count reductions.
                jk2 = pool_w.tile([P, 2, M], F16, tag="jk2")
                for c in range(1, C):
                    nc.scalar.activation(
                        out=jk2[:, 0, :], in_=eqs[c - 1][:], func=Act.Copy,
                        accum_out=accP[:, ch * 4 + c - 1 : ch * 4 + c])
                nc.scalar.activation(
                    out=jk2[:], in_=ip12[:], func=Act.Copy,
                    accum_out=accI[:, ch * 2 : ch * 2 + 1])
                nc.scalar.activation(
                    out=jk2[:], in_=ip34[:], func=Act.Copy,
                    accum_out=accI[:, ch * 2 + 1 : ch * 2 + 2])

            nc.sync.dma_start(out=yp[:], in_=accP[:])
            nc.sync.dma_start(out=yi[:], in_=accI[:])

    _legalize_waits(nc)
    return nc


def _get_program():
    if "nc" not in _prog_cache:
        _prog_cache["nc"] = _build_program()
    return _prog_cache["nc"]


def _run(input, target, trace=False, trace_kwargs=None):
    inp = np.asarray(input)
    tgt = np.asarray(target)
    assert inp.shape == (B, C, 128, 128, 128), inp.shape
    assert tgt.shape == (B, 128, 128, 128), tgt.shape

    inp_r = inp.reshape(B, C, N).astype(np.float16)
    tgt_r = tgt.reshape(B, N)

    in_maps = []
    t8s = []
    for core in range(NCORES):
        b, h = core // 2, core % 2
        xs = np.ascontiguousarray(inp_r[b, :, h * HALF : (h + 1) * HALF])
        ti = tgt_r[b, h * HALF : (h + 1) * HALF].astype(np.int8)
        t8s.append(ti)
        in_maps.append({"x": xs, "t": ti.astype(np.float16)})

    nc = _get_program()
    kw = {}
    if trace:
        kw["trace"] = True
        if trace_kwargs:
            kw.update(trace_kwargs)
    res = run_bass_kernel_spmd(nc, in_maps, list(range(NCORES)), **kw)

    # host combine: per (batch, class) counts from the two half-cores
    Pc = np.zeros((B, C), np.float64)
    Tc = np.zeros((B, C), np.float64)
    Ic = np.zeros((B, C), np.float64)
    for core in range(NCORES):
        b = core // 2
        r = res.results[core]
        Tc[b] += np.bincount(t8s[core], minlength=C)
        for c in range(1, C):
            cols = slice(c - 1, 4 * NCH, 4)
            Pc[b, c] += r["yp"][:, cols].sum()
        # unpack I pairs: col 2*ch is I1 + 4096*I2, col 2*ch+1 is I3 + 4096*I4
        ia = r["yi"].astype(np.float64)
        lo = np.mod(ia, PACK)
        hi = np.floor_divide(ia, PACK)
        Ic[b, 1] += lo[:, 0::2].sum()
        Ic[b, 2] += hi[:, 0::2].sum()
        Ic[b, 3] += lo[:, 1::2].sum()
        Ic[b, 4] += hi[:, 1::2].sum()

    inter = Ic[:, 1:].astype(np.float32)
    union = (Pc[:, 1:] + Tc[:, 1:]).astype(np.float32)
    dice = (2.0 * inter + np.float32(EPS)) / (union + np.float32(EPS))
    out = np.array([dice.mean(dtype=np.float32)], dtype=np.float32)
    return out, res


def kernel(input, target):
    out, _ = _run(input, target, trace=False)
    return out


# revision 7
# speedup vs baseline: 1.4392x; 1.1249x over previous
"""Dice-loss (segment_reduce) kernel for 8 Trainium2 NeuronCores.

Full inputs: input (4,5,128,128,128) f32, target (4,128,128,128) int64.
Output: scalar mean dice, shape (1,), f32 — matches the jax reference.

Sharding: 8 cores = 4 batches x 2 spatial halves. Each core computes, for
its 1,048,576 positions, per-class counts for classes 1..4:
  P_c = #(x_c == max over classes)        (argmax one-hot; exact ties
                                           overcount, tiny effect)
  I_c = #((x_c == max) and target == c)
Target-class counts T_c are exact and cheap on the host (np.bincount).
The host forms dice = (2I+eps)/(P+T+eps) and the final mean.

Engine plan (trace-driven for THIS toolchain):
  - fp16 host conversion: DVE tensor_tensor gets 2x mode, HBM halves.
  - STT runs 1x here (no 2x uop) -> all 2-source work is plain
    tensor_tensor (max / is_ge / mult, 2x); class masks are
    tensor_scalar (4x). DVE is the critical path at ~6 ops/position.
  - Counting runs on the TENSOR engine: ones[128,1] @ plane[128,512]
    matmuls partition-reduce every count plane into six [1,512] PSUM
    accumulators (4 P-classes, 2 packed I-pairs), accumulated across
    the whole kernel. ScalarE only copies the six rows to SBUF at the
    end. I-pairs pack class c (odd) and c+1 (even) as hit + 4096*hit;
    per-column totals stay < 2^24 so f32 accumulation is exact; the
    host decodes with mod/div and sums the 512 columns.
"""

import sys

sys.path.insert(0, "/opt/trn_rl_repo")

import numpy as np
import concourse.bass as bass
import concourse.mybir as mybir
from concourse.tile import TileContext
from concourse.bass_utils import run_bass_kernel_spmd

F32 = mybir.dt.float32
F16 = mybir.dt.float16
Alu = mybir.AluOpType
Act = mybir.ActivationFunctionType

B, C = 4, 5
N = 128 * 128 * 128          # spatial positions per batch
NCORES = 8
HALF = N // 2                # positions per core
P = 128                      # SBUF partitions
BLK = 512                    # PE moving-tensor free-dim block
# Ramped chunks (free-dim elems per partition, sum = HALF/P = 8192),
# each a multiple of BLK.
CHUNKS = (1024, 3584, 3584)
NCH = len(CHUNKS)
assert sum(CHUNKS) == HALF // P
assert all(m % BLK == 0 for m in CHUNKS)
PACK = 4096.0
EPS = 1e-5

_prog_cache = {}


def _legalize_waits(nc):
    """Split multi-wait instructions: this walrus build's codegen allows only
    one embedded sync-wait per instruction ("Too many sync wait commands").
    Move extra waits onto standalone EventSemaphore instructions inserted
    just before, on the same engine queue — semantically identical."""
    n_new = 0
    for bb in nc.main_func.blocks:
        insts = list(bb.instructions)
        out = []
        changed = False
        for ins in insts:
            si = ins.sync_info
            waits = list(si.on_wait) if si and si.on_wait else []
            if len(waits) > 1:
                for w in waits[:-1]:
                    ev = mybir.InstEventSemaphore(
                        name=f"legalw-{n_new}", ins=[], outs=[]
                    )
                    n_new += 1
                    ev.engine = ins.engine
                    ev.sync_info = mybir.SyncInfo(on_wait=[w], on_update=[])
                    nc.register_instruction(ev)
                    out.append(ev)
                ins.sync_info = mybir.SyncInfo(
                    on_wait=[waits[-1]], on_update=list(si.on_update or [])
                )
                changed = True
            out.append(ins)
        if changed:
            live = bb.instructions
            live.clear()
            live.extend(out)
    return n_new


def _build_program():
    nc = bass.Bass()

    x = nc.dram_tensor("x", [C, HALF], F16, kind="ExternalInput")
    t = nc.dram_tensor("t", [HALF], F16, kind="ExternalInput")
    # 6 reduced rows of 512: P1..P4, Ipair12, Ipair34
    yc = nc.dram_tensor("yc", [1, 6 * BLK], F32, kind="ExternalOutput")

    xr = x[:].rearrange("c (p f) -> p c f", p=P)
    tr = t[:].rearrange("(p f) -> p f", p=P)

    nblk_tot = sum(m // BLK for m in CHUNKS)

    with TileContext(nc) as tc:
        with (
            tc.tile_pool(name="xin", bufs=2) as pool_x,
            tc.tile_pool(name="tin", bufs=2) as pool_t,
            tc.tile_pool(name="work", bufs=1) as pool_w,
            tc.tile_pool(name="ones", bufs=1) as pool_o,
            tc.tile_pool(name="psum", bufs=1, space="PSUM") as pool_p,
            tc.tile_pool(name="accs", bufs=1) as pool_a,
        ):
            ones = pool_o.tile([P, 1], F16)
            nc.vector.memset(ones[:], 1.0)

            # six psum accumulators, each one bank
            ps = [pool_p.tile([1, BLK], F32, name=f"ps{i}") for i in range(6)]
            accO = pool_a.tile([1, 6 * BLK], F32)

            blk_idx = 0  # running block counter to place start/stop flags
            off = 0
            for ch, M in enumerate(CHUNKS):
                nb = M // BLK
                xt = pool_x.tile([P, C, M], F16, tag="xt")
                tt = pool_t.tile([P, M], F16, tag="tt")
                nc.sync.dma_start(out=xt[:, 0:2, :], in_=xr[:, 0:2, off : off + M])
                nc.sync.dma_start(out=xt[:, 2:5, :], in_=xr[:, 2:5, off : off + M])
                nc.sync.dma_start(out=tt[:], in_=tr[:, off : off + M])
                off += M

                # DVE: max over the 5 classes (tree), fp16 2x mode.
                ma = pool_w.tile([P, M], F16, tag="ma")
                mb = pool_w.tile([P, M], F16, tag="mb")
                mc_ = pool_w.tile([P, M], F16, tag="mc")
                mx = pool_w.tile([P, M], F16, tag="ma", name=f"mx_{ch}")
                nc.vector.tensor_tensor(out=ma[:], in0=xt[:, 0, :], in1=xt[:, 1, :], op=Alu.max)
                nc.vector.tensor_tensor(out=mb[:], in0=xt[:, 2, :], in1=xt[:, 3, :], op=Alu.max)
                nc.vector.tensor_tensor(out=mc_[:], in0=ma[:], in1=mb[:], op=Alu.max)
                nc.vector.tensor_tensor(out=mx[:], in0=mc_[:], in1=xt[:, 4, :], op=Alu.max)

                first = blk_idx == 0
                last_of = blk_idx + nb == nblk_tot

                # DVE: eq_c = (x_c >= M) (TT is_ge, 2x); PE partition-reduces
                # each 512-block into psum P_c.
                eqs = []
                for c in range(1, C):
                    eq = pool_w.tile([P, M], F16, tag=f"eq{c}", name=f"eq{c}_{ch}")
                    nc.vector.tensor_tensor(
                        out=eq[:], in0=xt[:, c, :], in1=mx[:], op=Alu.is_ge)
                    eqs.append(eq)
                    for b in range(nb):
                        nc.tensor.matmul(
                            out=ps[c - 1][:],
                            lhsT=ones[:],
                            rhs=eq[:, b * BLK : (b + 1) * BLK],
                            start=(first and b == 0),
                            stop=(last_of and b == nb - 1),
                        )

                # DVE: class masks (TS, 4x; even class pre-scaled by 4096)
                # and masked hits (TT mult, 2x) into paired [P,2,M] tiles.
                ip12 = pool_w.tile([P, 2, M], F16, tag="ip12")
                ip34 = pool_w.tile([P, 2, M], F16, tag="ip34")
                ips = [ip12[:, 0, :], ip12[:, 1, :], ip34[:, 0, :], ip34[:, 1, :]]
                pair_of = [0, 0, 1, 1]
                for c in range(1, C):
                    h = pool_w.tile([P, M], F16, tag=f"h{c % 2}", name=f"h{c}_{ch}")
                    if c % 2 == 0:
                        nc.vector.tensor_scalar(
                            out=h[:], in0=tt[:], scalar1=float(c), scalar2=PACK,
                            op0=Alu.is_equal, op1=Alu.mult)
                    else:
                        nc.vector.tensor_scalar(
                            out=h[:], in0=tt[:], scalar1=float(c), scalar2=None,
                            op0=Alu.is_equal)
                    nc.vector.tensor_tensor(
                        out=ips[c - 1], in0=eqs[c - 1][:], in1=h[:], op=Alu.mult)
                    for b in range(nb):
                        nc.tensor.matmul(
                            out=ps[4 + pair_of[c - 1]][:],
                            lhsT=ones[:],
                            rhs=ips[c - 1][:, b * BLK : (b + 1) * BLK],
                            start=(first and b == 0 and c % 2 == 1),
                            stop=(last_of and b == nb - 1 and c % 2 == 0),
                        )

                blk_idx += nb

            # ACT: copy the six psum rows to SBUF, then DMA out.
            for i in range(6):
                nc.scalar.activation(
                    out=accO[:, i * BLK : (i + 1) * BLK], in_=ps[i][:],
                    func=Act.Copy)
            nc.sync.dma_start(out=yc[:], in_=accO[:])

    _legalize_waits(nc)
    return nc


def _get_program():
    if "nc" not in _prog_cache:
        _prog_cache["nc"] = _build_program()
    return _prog_cache["nc"]


def _run(input, target, trace=False, trace_kwargs=None):
    inp = np.asarray(input)
    tgt = np.asarray(target)
    assert inp.shape == (B, C, 128, 128, 128), inp.shape
    assert tgt.shape == (B, 128, 128, 128), tgt.shape

    inp_r = inp.reshape(B, C, N).astype(np.float16)
    tgt_r = tgt.reshape(B, N)

    in_maps = []
    t8s = []
    for core in range(NCORES):
        b, h = core // 2, core % 2
        xs = np.ascontiguousarray(inp_r[b, :, h * HALF : (h + 1) * HALF])
        ti = tgt_r[b, h * HALF : (h + 1) * HALF].astype(np.int8)
        t8s.append(ti)
        in_maps.append({"x": xs, "t": ti.astype(np.float16)})

    nc = _get_program()
    kw = {}
    if trace:
        kw["trace"] = True
        if trace_kwargs:
            kw.update(trace_kwargs)
    res = run_bass_kernel_spmd(nc, in_maps, list(range(NCORES)), **kw)

    # host combine: per (batch, class) counts from the two half-cores
    Pc = np.zeros((B, C), np.float64)
    Tc = np.zeros((B, C), np.float64)
    Ic = np.zeros((B, C), np.float64)
    for core in range(NCORES):
        b = core // 2
        r = res.results[core]
        Tc[b] += np.bincount(t8s[core], minlength=C)
        yv = r["yc"].reshape(6, 512).astype(np.float64)
        for c in range(1, C):
            Pc[b, c] += yv[c - 1].sum()
        for pair, (clo, chi) in enumerate([(1, 2), (3, 4)]):
            a = yv[4 + pair]
            Ic[b, clo] += np.mod(a, PACK).sum()
            Ic[b, chi] += np.floor_divide(a, PACK).sum()

    inter = Ic[:, 1:].astype(np.float32)
    union = (Pc[:, 1:] + Tc[:, 1:]).astype(np.float32)
    dice = (2.0 * inter + np.float32(EPS)) / (union + np.float32(EPS))
    out = np.array([dice.mean(dtype=np.float32)], dtype=np.float32)
    return out, res


def kernel(input, target):
    out, _ = _run(input, target, trace=False)
    return out


# revision 9
# speedup vs baseline: 1.5674x; 1.0890x over previous
"""Dice-loss (segment_reduce) kernel for 8 Trainium2 NeuronCores.

Full inputs: input (4,5,128,128,128) f32, target (4,128,128,128) int64.
Output: scalar mean dice, shape (1,), f32 — matches the jax reference.

Sharding: 8 cores = 4 batches x 2 spatial halves. Each core computes, for
its 1,048,576 positions, per-class counts for classes 1..4:
  P_c = #(x_c == max over classes)        (argmax one-hot; exact ties
                                           overcount, tiny effect)
  I_c = #((x_c == max) and target == c)
Target-class counts T_c are exact and cheap on the host (np.bincount).
The host forms dice = (2I+eps)/(P+T+eps) and the final mean.

Engine plan (trace-driven for THIS toolchain):
  - fp16 host conversion: DVE tensor_tensor runs in 2x packed mode and
    HBM traffic halves. STT/TS-with-accum run 1x here, so the DVE does
    ONLY plain tensor_tensor (max tree 4, is_ge 4, masked-mult 4 =
    6 cycles/position), which is the 2-read-port floor for this
    dataflow.
  - The target arrives as two packed mask planes made on the host:
    w12 = [t==1] + 4096*[t==2], w34 = [t==3] + 4096*[t==4] (fp16-exact
    values {0,1,4096}). eq_c * wpair then carries class c's hits in the
    mod-4096 residue and the partner class's hits in the 4096 multiple;
    the host separates them after the final reduction.
  - Counting runs on the TENSOR engine: ones[128,1] @ plane[128,512]
    matmuls partition-reduce each count plane into eight [1,512] PSUM
    accumulators (4 P-classes, 4 I-classes), accumulated across the
    whole kernel (per-column totals < 2^24, so f32 stays exact).
    ScalarE only copies the eight rows to SBUF at the end.
"""

import sys

sys.path.insert(0, "/opt/trn_rl_repo")

import numpy as np
import concourse.bass as bass
import concourse.mybir as mybir
from concourse.tile import TileContext
from concourse.bass_utils import run_bass_kernel_spmd

F32 = mybir.dt.float32
F16 = mybir.dt.float16
Alu = mybir.AluOpType
Act = mybir.ActivationFunctionType

B, C = 4, 5
N = 128 * 128 * 128          # spatial positions per batch
NCORES = 8
HALF = N // 2                # positions per core
P = 128                      # SBUF partitions
BLK = 512                    # PE moving-tensor free-dim block
# Ramped chunks (free-dim elems per partition, sum = HALF/P = 8192),
# each a multiple of BLK.
CHUNKS = (1024, 3584, 3584)
NCH = len(CHUNKS)
assert sum(CHUNKS) == HALF // P
assert all(m % BLK == 0 for m in CHUNKS)
PACK = 4096.0
EPS = 1e-5

_prog_cache = {}


def _legalize_waits(nc):
    """Split multi-wait instructions: this walrus build's codegen allows only
    one embedded sync-wait per instruction ("Too many sync wait commands").
    Move extra waits onto standalone EventSemaphore instructions inserted
    just before, on the same engine queue — semantically identical."""
    n_new = 0
    for bb in nc.main_func.blocks:
        insts = list(bb.instructions)
        out = []
        changed = False
        for ins in insts:
            si = ins.sync_info
            waits = list(si.on_wait) if si and si.on_wait else []
            if len(waits) > 1:
                for w in waits[:-1]:
                    ev = mybir.InstEventSemaphore(
                        name=f"legalw-{n_new}", ins=[], outs=[]
                    )
                    n_new += 1
                    ev.engine = ins.engine
                    ev.sync_info = mybir.SyncInfo(on_wait=[w], on_update=[])
                    nc.register_instruction(ev)
                    out.append(ev)
                ins.sync_info = mybir.SyncInfo(
                    on_wait=[waits[-1]], on_update=list(si.on_update or [])
                )
                changed = True
            out.append(ins)
        if changed:
            live = bb.instructions
            live.clear()
            live.extend(out)
    return n_new


def _build_program():
    nc = bass.Bass()

    x = nc.dram_tensor("x", [C, HALF], F16, kind="ExternalInput")
    w = nc.dram_tensor("w", [2, HALF], F16, kind="ExternalInput")
    # 8 reduced rows of 512: P1..P4, I1..I4 (I rows need mod/div decode)
    yc = nc.dram_tensor("yc", [1, 8 * BLK], F32, kind="ExternalOutput")

    xr = x[:].rearrange("c (p f) -> p c f", p=P)
    wr = w[:].rearrange("c (p f) -> p c f", p=P)

    nblk_tot = sum(m // BLK for m in CHUNKS)

    with TileContext(nc) as tc:
        with (
            tc.tile_pool(name="xin", bufs=2) as pool_x,
            tc.tile_pool(name="win", bufs=2) as pool_t,
            tc.tile_pool(name="work", bufs=1) as pool_w,
            tc.tile_pool(name="ones", bufs=1) as pool_o,
            tc.tile_pool(name="psum", bufs=1, space="PSUM") as pool_p,
            tc.tile_pool(name="accs", bufs=1) as pool_a,
        ):
            ones = pool_o.tile([P, 1], F16)
            nc.vector.memset(ones[:], 1.0)

            # eight psum accumulators, one bank each: P1..P4, I1..I4
            ps = [pool_p.tile([1, BLK], F32, name=f"ps{i}") for i in range(8)]
            accO = pool_a.tile([1, 8 * BLK], F32)

            blk_idx = 0
            off = 0
            for ch, M in enumerate(CHUNKS):
                nb = M // BLK
                xt = pool_x.tile([P, C, M], F16, tag="xt")
                wt = pool_t.tile([P, 2, M], F16, tag="wt")
                nc.sync.dma_start(out=xt[:, 0:2, :], in_=xr[:, 0:2, off : off + M])
                nc.sync.dma_start(out=xt[:, 2:5, :], in_=xr[:, 2:5, off : off + M])
                nc.sync.dma_start(out=wt[:], in_=wr[:, :, off : off + M])
                off += M

                first = blk_idx == 0
                last_of = blk_idx + nb == nblk_tot

                # DVE: max over the 5 classes (tree), fp16 2x mode.
                ma = pool_w.tile([P, M], F16, tag="ma")
                mb = pool_w.tile([P, M], F16, tag="mb")
                mc_ = pool_w.tile([P, M], F16, tag="mc")
                mx = pool_w.tile([P, M], F16, tag="ma", name=f"mx_{ch}")
                nc.vector.tensor_tensor(out=ma[:], in0=xt[:, 0, :], in1=xt[:, 1, :], op=Alu.max)
                nc.vector.tensor_tensor(out=mb[:], in0=xt[:, 2, :], in1=xt[:, 3, :], op=Alu.max)
                nc.vector.tensor_tensor(out=mc_[:], in0=ma[:], in1=mb[:], op=Alu.max)
                nc.vector.tensor_tensor(out=mx[:], in0=mc_[:], in1=xt[:, 4, :], op=Alu.max)

                # Per class: eq_c (TT is_ge) -> PE P-count; eq_c * wpair
                # (TT mult) -> PE I-count. All DVE ops 2x, all counting PE.
                for c in range(1, C):
                    eq = pool_w.tile([P, M], F16, tag=f"eq{c}", name=f"eq{c}_{ch}")
                    nc.vector.tensor_tensor(
                        out=eq[:], in0=xt[:, c, :], in1=mx[:], op=Alu.is_ge)
                    for b in range(nb):
                        nc.tensor.matmul(
                            out=ps[c - 1][:],
                            lhsT=ones[:],
                            rhs=eq[:, b * BLK : (b + 1) * BLK],
                            start=(first and b == 0),
                            stop=(last_of and b == nb - 1),
                        )
                    ip = pool_w.tile([P, M], F16, tag=f"ip{c}", name=f"ip{c}_{ch}")
                    nc.vector.tensor_tensor(
                        out=ip[:], in0=eq[:], in1=wt[:, (c - 1) // 2, :], op=Alu.mult)
                    for b in range(nb):
                        nc.tensor.matmul(
                            out=ps[4 + c - 1][:],
                            lhsT=ones[:],
                            rhs=ip[:, b * BLK : (b + 1) * BLK],
                            start=(first and b == 0),
                            stop=(last_of and b == nb - 1),
                        )

                blk_idx += nb

            # ACT: copy the eight psum rows to SBUF, then DMA out.
            for i in range(8):
                nc.scalar.activation(
                    out=accO[:, i * BLK : (i + 1) * BLK], in_=ps[i][:],
                    func=Act.Copy)
            nc.sync.dma_start(out=yc[:], in_=accO[:])

    _legalize_waits(nc)
    return nc


def _get_program():
    if "nc" not in _prog_cache:
        _prog_cache["nc"] = _build_program()
    return _prog_cache["nc"]


def _run(input, target, trace=False, trace_kwargs=None):
    inp = np.asarray(input)
    tgt = np.asarray(target)
    assert inp.shape == (B, C, 128, 128, 128), inp.shape
    assert tgt.shape == (B, 128, 128, 128), tgt.shape

    inp_r = inp.reshape(B, C, N).astype(np.float16)
    tgt_r = tgt.reshape(B, N).astype(np.int8)

    in_maps = []
    t8s = []
    for core in range(NCORES):
        b, h = core // 2, core % 2
        xs = np.ascontiguousarray(inp_r[b, :, h * HALF : (h + 1) * HALF])
        ti = tgt_r[b, h * HALF : (h + 1) * HALF]
        t8s.append(ti)
        wv = np.empty((2, HALF), np.float16)
        wv[0] = (ti == 1).astype(np.float16)
        wv[0] += np.float16(PACK) * (ti == 2).astype(np.float16)
        wv[1] = (ti == 3).astype(np.float16)
        wv[1] += np.float16(PACK) * (ti == 4).astype(np.float16)
        in_maps.append({"x": xs, "w": wv})

    nc = _get_program()
    kw = {}
    if trace:
        kw["trace"] = True
        if trace_kwargs:
            kw.update(trace_kwargs)
    res = run_bass_kernel_spmd(nc, in_maps, list(range(NCORES)), **kw)

    # host combine: per (batch, class) counts from the two half-cores
    Pc = np.zeros((B, C), np.float64)
    Tc = np.zeros((B, C), np.float64)
    Ic = np.zeros((B, C), np.float64)
    for core in range(NCORES):
        b = core // 2
        r = res.results[core]
        Tc[b] += np.bincount(t8s[core], minlength=C)
        yv = r["yc"].reshape(8, BLK).astype(np.float64)
        for c in range(1, C):
            Pc[b, c] += yv[c - 1].sum()
            a = yv[4 + c - 1]
            if c % 2 == 1:
                Ic[b, c] += np.mod(a, PACK).sum()
            else:
                Ic[b, c] += np.floor_divide(a, PACK).sum()

    inter = Ic[:, 1:].astype(np.float32)
    union = (Pc[:, 1:] + Tc[:, 1:]).astype(np.float32)
    dice = (2.0 * inter + np.float32(EPS)) / (union + np.float32(EPS))
    out = np.array([dice.mean(dtype=np.float32)], dtype=np.float32)
    return out, res


def kernel(input, target):
    out, _ = _run(input, target, trace=False)
    return out


# revision 10
# speedup vs baseline: 1.6638x; 1.0615x over previous
"""Dice-loss (segment_reduce) kernel for 8 Trainium2 NeuronCores.

Full inputs: input (4,5,128,128,128) f32, target (4,128,128,128) int64.
Output: scalar mean dice, shape (1,), f32 — matches the jax reference.

Sharding: 8 cores = 4 batches x 2 spatial halves. Each core computes, for
its 1,048,576 positions, per-class counts for classes 1..4:
  P_c = #(x_c == max over classes)        (argmax one-hot; exact ties
                                           overcount, tiny effect)
  I_c = #((x_c == max) and target == c)
Target-class counts T_c are exact and cheap on the host (np.bincount).
The host forms dice = (2I+eps)/(P+T+eps) and the final mean.

Engine plan (trace-driven for THIS toolchain):
  - fp16 host conversion: DVE tensor_tensor runs in 2x packed mode and
    HBM traffic halves. STT/TS-with-accum run 1x here, so the DVE does
    ONLY plain tensor_tensor (max tree 4, is_ge 4, masked-mult 4 =
    6 cycles/position), which is the 2-read-port floor for this
    dataflow.
  - The target arrives as two packed mask planes made on the host:
    w12 = [t==1] + 4096*[t==2], w34 = [t==3] + 4096*[t==4] (fp16-exact
    values {0,1,4096}). eq_c * wpair then carries class c's hits in the
    mod-4096 residue and the partner class's hits in the 4096 multiple;
    the host separates them after the final reduction.
  - Counting runs on the TENSOR engine: ones[128,1] @ plane[128,512]
    matmuls partition-reduce each count plane into eight [1,512] PSUM
    accumulators (4 P-classes, 4 I-classes), accumulated across the
    whole kernel (per-column totals < 2^24, so f32 stays exact).
    ScalarE only copies the eight rows to SBUF at the end.
"""

import sys

sys.path.insert(0, "/opt/trn_rl_repo")

import numpy as np
import concourse.bass as bass
import concourse.mybir as mybir
from concourse.tile import TileContext
from concourse.bass_utils import run_bass_kernel_spmd

F32 = mybir.dt.float32
F16 = mybir.dt.float16
Alu = mybir.AluOpType
Act = mybir.ActivationFunctionType

B, C = 4, 5
N = 128 * 128 * 128          # spatial positions per batch
NCORES = 8
HALF = N // 2                # positions per core
P = 128                      # SBUF partitions
BLK = 512                    # PE moving-tensor free-dim block
# Ramped chunks (free-dim elems per partition, sum = HALF/P = 8192),
# each a multiple of BLK.
CHUNKS = (512, 1536, 3072, 3072)
NCH = len(CHUNKS)
assert sum(CHUNKS) == HALF // P
assert all(m % BLK == 0 for m in CHUNKS)
PACK = 4096.0
EPS = 1e-5

_prog_cache = {}


def _legalize_waits(nc):
    """Split multi-wait instructions: this walrus build's codegen allows only
    one embedded sync-wait per instruction ("Too many sync wait commands").
    Move extra waits onto standalone EventSemaphore instructions inserted
    just before, on the same engine queue — semantically identical."""
    n_new = 0
    for bb in nc.main_func.blocks:
        insts = list(bb.instructions)
        out = []
        changed = False
        for ins in insts:
            si = ins.sync_info
            waits = list(si.on_wait) if si and si.on_wait else []
            if len(waits) > 1:
                for w in waits[:-1]:
                    ev = mybir.InstEventSemaphore(
                        name=f"legalw-{n_new}", ins=[], outs=[]
                    )
                    n_new += 1
                    ev.engine = ins.engine
                    ev.sync_info = mybir.SyncInfo(on_wait=[w], on_update=[])
                    nc.register_instruction(ev)
                    out.append(ev)
                ins.sync_info = mybir.SyncInfo(
                    on_wait=[waits[-1]], on_update=list(si.on_update or [])
                )
                changed = True
            out.append(ins)
        if changed:
            live = bb.instructions
            live.clear()
            live.extend(out)
    return n_new


def _build_program():
    nc = bass.Bass()

    x = nc.dram_tensor("x", [C, HALF], F16, kind="ExternalInput")
    w = nc.dram_tensor("w", [2, HALF], F16, kind="ExternalInput")
    # 8 reduced rows of 512: P1..P4, I1..I4 (I rows need mod/div decode)
    yc = nc.dram_tensor("yc", [1, 8 * BLK], F32, kind="ExternalOutput")

    xr = x[:].rearrange("c (p f) -> p c f", p=P)
    wr = w[:].rearrange("c (p f) -> p c f", p=P)

    nblk_tot = sum(m // BLK for m in CHUNKS)

    with TileContext(nc) as tc:
        with (
            tc.tile_pool(name="xin", bufs=2) as pool_x,
            tc.tile_pool(name="win", bufs=2) as pool_t,
            tc.tile_pool(name="work", bufs=1) as pool_w,
            tc.tile_pool(name="ones", bufs=1) as pool_o,
            tc.tile_pool(name="psum", bufs=1, space="PSUM") as pool_p,
            tc.tile_pool(name="accs", bufs=1) as pool_a,
        ):
            ones = pool_o.tile([P, 1], F16)
            nc.vector.memset(ones[:], 1.0)

            # eight psum accumulators, one bank each: P1..P4, I1..I4
            ps = [pool_p.tile([1, BLK], F32, name=f"ps{i}") for i in range(8)]
            accO = pool_a.tile([1, 8 * BLK], F32)

            blk_idx = 0
            off = 0
            for ch, M in enumerate(CHUNKS):
                nb = M // BLK
                xt = pool_x.tile([P, C, M], F16, tag="xt")
                wt = pool_t.tile([P, 2, M], F16, tag="wt")
                nc.sync.dma_start(out=xt[:, 0:2, :], in_=xr[:, 0:2, off : off + M])
                nc.sync.dma_start(out=xt[:, 2:4, :], in_=xr[:, 2:4, off : off + M])
                nc.sync.dma_start(out=xt[:, 4:5, :], in_=xr[:, 4:5, off : off + M])
                nc.sync.dma_start(out=wt[:], in_=wr[:, :, off : off + M])
                off += M

                first = blk_idx == 0
                last_of = blk_idx + nb == nblk_tot

                # DVE: max over the 5 classes (tree), fp16 2x mode.
                ma = pool_w.tile([P, M], F16, tag="ma")
                mb = pool_w.tile([P, M], F16, tag="mb")
                mc_ = pool_w.tile([P, M], F16, tag="mc")
                mx = pool_w.tile([P, M], F16, tag="ma", name=f"mx_{ch}")
                nc.vector.tensor_tensor(out=ma[:], in0=xt[:, 0, :], in1=xt[:, 1, :], op=Alu.max)
                nc.vector.tensor_tensor(out=mb[:], in0=xt[:, 2, :], in1=xt[:, 3, :], op=Alu.max)
                nc.vector.tensor_tensor(out=mc_[:], in0=ma[:], in1=mb[:], op=Alu.max)
                nc.vector.tensor_tensor(out=mx[:], in0=mc_[:], in1=xt[:, 4, :], op=Alu.max)

                # Per class: eq_c (TT is_ge) -> PE P-count; eq_c * wpair
                # (TT mult) -> PE I-count. All DVE ops 2x, all counting PE.
                for c in range(1, C):
                    eq = pool_w.tile([P, M], F16, tag=f"eq{c}", name=f"eq{c}_{ch}")
                    nc.vector.tensor_tensor(
                        out=eq[:], in0=xt[:, c, :], in1=mx[:], op=Alu.is_ge)
                    for b in range(nb):
                        nc.tensor.matmul(
                            out=ps[c - 1][:],
                            lhsT=ones[:],
                            rhs=eq[:, b * BLK : (b + 1) * BLK],
                            start=(first and b == 0),
                            stop=(last_of and b == nb - 1),
                        )
                    ip = pool_w.tile([P, M], F16, tag=f"ip{c}", name=f"ip{c}_{ch}")
                    if last_of and c == C - 1:
                        hm = M // 2
                        nc.vector.tensor_tensor(
                            out=ip[:, 0:hm], in0=eq[:, 0:hm],
                            in1=wt[:, (c - 1) // 2, 0:hm], op=Alu.mult)
                        nc.vector.tensor_tensor(
                            out=ip[:, hm:M], in0=eq[:, hm:M],
                            in1=wt[:, (c - 1) // 2, hm:M], op=Alu.mult)
                    else:
                        nc.vector.tensor_tensor(
                            out=ip[:], in0=eq[:], in1=wt[:, (c - 1) // 2, :], op=Alu.mult)
                    for b in range(nb):
                        nc.tensor.matmul(
                            out=ps[4 + c - 1][:],
                            lhsT=ones[:],
                            rhs=ip[:, b * BLK : (b + 1) * BLK],
                            start=(first and b == 0),
                            stop=(last_of and b == nb - 1),
                        )

                blk_idx += nb

            # ACT: copy the eight psum rows to SBUF, then DMA out.
            for i in range(8):
                nc.scalar.activation(
                    out=accO[:, i * BLK : (i + 1) * BLK], in_=ps[i][:],
                    func=Act.Copy)
            nc.sync.dma_start(out=yc[:], in_=accO[:])

    _legalize_waits(nc)
    return nc


def _get_program():
    if "nc" not in _prog_cache:
        _prog_cache["nc"] = _build_program()
    return _prog_cache["nc"]


def _run(input, target, trace=False, trace_kwargs=None):
    inp = np.asarray(input)
    tgt = np.asarray(target)
    assert inp.shape == (B, C, 128, 128, 128), inp.shape
    assert tgt.shape == (B, 128, 128, 128), tgt.shape

    inp_r = inp.reshape(B, C, N).astype(np.float16)
    tgt_r = tgt.reshape(B, N).astype(np.int8)

    in_maps = []
    t8s = []
    for core in range(NCORES):
        b, h = core // 2, core % 2
        xs = np.ascontiguousarray(inp_r[b, :, h * HALF : (h + 1) * HALF])
        ti = tgt_r[b, h * HALF : (h + 1) * HALF]
        t8s.append(ti)
        wv = np.empty((2, HALF), np.float16)
        wv[0] = (ti == 1).astype(np.float16)
        wv[0] += np.float16(PACK) * (ti == 2).astype(np.float16)
        wv[1] = (ti == 3).astype(np.float16)
        wv[1] += np.float16(PACK) * (ti == 4).astype(np.float16)
        in_maps.append({"x": xs, "w": wv})

    nc = _get_program()
    kw = {}
    if trace:
        kw["trace"] = True
        if trace_kwargs:
            kw.update(trace_kwargs)
    res = run_bass_kernel_spmd(nc, in_maps, list(range(NCORES)), **kw)

    # host combine: per (batch, class) counts from the two half-cores
    Pc = np.zeros((B, C), np.float64)
    Tc = np.zeros((B, C), np.float64)
    Ic = np.zeros((B, C), np.float64)
    for core in range(NCORES):
        b = core // 2
        r = res.results[core]
        Tc[b] += np.bincount(t8s[core], minlength=C)
        yv = r["yc"].reshape(8, BLK).astype(np.float64)
        for c in range(1, C):
            Pc[b, c] += yv[c - 1].sum()
            a = yv[4 + c - 1]
            if c % 2 == 1:
                Ic[b, c] += np.mod(a, PACK).sum()
            else:
                Ic[b, c] += np.floor_divide(a, PACK).sum()

    inter = Ic[:, 1:].astype(np.float32)
    union = (Pc[:, 1:] + Tc[:, 1:]).astype(np.float32)
    dice = (2.0 * inter + np.float32(EPS)) / (union + np.float32(EPS))
    out = np.array([dice.mean(dtype=np.float32)], dtype=np.float32)
    return out, res


def kernel(input, target):
    out, _ = _run(input, target, trace=False)
    return out
